# revision 3
# baseline (speedup 1.0000x reference)
"""Trainium2 Bass kernel for the 5-head detection tower (nn_DFD_10849087390476).

Network (per head h of 5): 1x1-conv tower on x [B,64,H,W]:
    h1 = relu(Win x + bin)
    h2 = h1 + relu(Wh0 h1 + bh0)
    h3 = h2 + relu(Wh1 h2 + bh1)
    out_h = Wout h3 + bout
Output = concat over heads: channels [cls 81, obj 2, box 4, pos 64, ins 128] = 279.

Sharding: data-parallel over (batch, H/2) -> 8 shards of 32768 pixels.
Per core the pixel set is split in two 16384-px groups (A on SBUF partitions
0-63, B on 64-127). Heads are paired (cls+obj, box+pos) with block-diagonal
128x128 stationaries so every matmul uses the full partition dim; the ins head
rides the A/B stacking instead. Matmuls run in float32r (TF32-like, 1 cycle/row
at free-dim 512). Residuals are fused on the vector engine as max(psum,0)+h via
scalar_tensor_tensor; in-layer bias comes free via the scalar-engine activation
bias, out-layer bias via activation Identity+bias.

DMA layout: one packed weight DMA; x loaded in [128,4096] chunks; outputs
staged in SBUF for 8 pixel-tiles and flushed as [mo,4096] stores with
16KB-contiguous per-partition runs (keeps HWDGE descriptor count low).
"""
import numpy as np

from concourse import bacc, tile
import concourse.mybir as mybir
from concourse.bass_utils import run_bass_kernel_spmd

F32 = mybir.dt.float32
F32R = mybir.dt.float32r
AF = mybir.ActivationFunctionType
ALU = mybir.AluOpType

B, C, H, W = 4, 64, 256, 256
NCORES = 8
NPX = (B * H * W) // NCORES          # 32768 pixels per core
NG = NPX // 2                        # 16384 per group (A/B)
T = 512                              # pixels per matmul tile
NT = NG // T                         # 32 pair-tiles per core
GT = 8                               # pair-tiles per output flush group
NF = NT // GT                        # 4 flush groups
OD = 279                             # output channels

SECS = ("co", "bp", "ins")
MO = {"co": 83, "bp": 68, "ins": 128}
OCH = {"co": (0, 83), "bp": (83, 151), "ins": (151, 279)}

# packed weight-tensor column layout
_W_COLS = {}
_c = 0
for _n in ("sin_co", "sin_bp", "sin_ins", "sl1_co", "sl1_bp", "sl1_ins",
           "sl2_co", "sl2_bp", "sl2_ins"):
    _W_COLS[_n] = (_c, 128); _c += 128
for _s in SECS:
    _W_COLS["sout_" + _s] = (_c, MO[_s]); _c += MO[_s]
for _n in ("bin_co", "bin_bp", "bin_ins",
           "bh1_co", "bh1_bp", "bh1_ins", "bh2_co", "bh2_bp", "bh2_ins",
           "bout_co", "bout_bp", "bout_ins"):
    _W_COLS[_n] = (_c, 1); _c += 1
WCOLS_TOTAL = _c

_last_results = None                 # test.py reads exec_time_ns from here
_cache = {}


def _bd(a, b):
    out = np.zeros((a.shape[0] + b.shape[0], a.shape[1] + b.shape[1]), np.float32)
    out[:a.shape[0], :a.shape[1]] = a
    out[a.shape[0]:, a.shape[1]:] = b
    return out


def _build(fast: bool):
    nc = bacc.Bacc("TRN2", target_bir_lowering=False, debug=False)

    xs_d = nc.dram_tensor("xs", [128, NG], F32, kind="ExternalInput")
    wp_d = nc.dram_tensor("wp", [128, WCOLS_TOTAL], F32, kind="ExternalInput")
    out_d = nc.dram_tensor("out", [OD, NPX], F32, kind="ExternalOutput")

    with tile.TileContext(nc) as tc:
        with tc.tile_pool(name="const", bufs=1) as cpool, \
             tc.tile_pool(name="xp", bufs=2) as xpool, \
             tc.tile_pool(name="hp", bufs=3) as hpool, \
             tc.tile_pool(name="op", bufs=1) as opool, \
             tc.tile_pool(name="ps", bufs=2, space="PSUM") as pspool, \
             tc.tile_pool(name="pso", bufs=2, space="PSUM") as psopool:

            wp_t = cpool.tile([128, WCOLS_TOTAL], F32R, tag="wp")
            nc.sync.dma_start(out=wp_t[:], in_=wp_d.ap().bitcast(F32R))

            def wap(name):                      # stationary weight AP (f32r)
                c0, n = _W_COLS[name]
                return wp_t[:, c0:c0 + n]

            def bap(name, rows=128):            # bias AP (f32)
                c0, _ = _W_COLS[name]
                return wp_t[0:rows, c0:c0 + 1].bitcast(F32)

            def residual(h_prev, psum, l, s):
                """h_next = h_prev + relu(psum + bh)."""
                P, FD = h_prev.shape[0], h_prev.shape[1]
                h_next = hpool.tile([P, FD], F32R, tag=f"h{s}")
                if fast:
                    nc.vector.scalar_tensor_tensor(
                        h_next[:], psum[:], 0.0, h_prev[:], ALU.max, ALU.add)
                else:
                    r = hpool.tile([P, FD], F32, tag=f"r{s}")
                    nc.scalar.activation(r[:], psum[:], AF.Relu,
                                         bias=bap(f"bh{l}_{s}"), scale=1.0)
                    nc.vector.tensor_add(h_next[:], h_prev[:], r[:])
                return h_next

            for f in range(NF):
                x_t = xpool.tile([128, GT * T], F32R, tag="x")
                nc.sync.dma_start(
                    out=x_t[:],
                    in_=xs_d.ap()[:, f * GT * T:(f + 1) * GT * T].bitcast(F32R))

                o_t = {(s, seg): opool.tile([MO[s], GT * T], F32,
                                            tag=f"o{s}{seg}", name=f"o_{s}_{seg}")
                       for s in SECS for seg in (0, 1)}

                for gl in range(GT):
                    xg = x_t[:, gl * T:(gl + 1) * T]
                    for s in SECS:
                        mo = MO[s]
                        # ---- in-proj ----
                        if s == "ins":
                            ps_in = pspool.tile([128, T], F32, tag="ps")
                            nc.tensor.matmul(ps_in[:], wap("sin_ins"), xg,
                                             start=True, stop=True)
                            h1 = hpool.tile([128, T], F32R, tag="hins")
                            nc.scalar.activation(h1[:], ps_in[:], AF.Relu,
                                                 bias=bap("bin_ins"), scale=1.0)
                        else:
                            ps_in = pspool.tile([128, 2 * T], F32, tag="ps")
                            nc.tensor.matmul(ps_in[:, 0:T], wap("sin_" + s)[0:64, :],
                                             xg[0:64, :], start=True, stop=True)
                            nc.tensor.matmul(ps_in[:, T:2 * T],
                                             wap("sin_" + s)[64:128, :],
                                             xg[64:128, :], start=True, stop=True)
                            h1 = hpool.tile([128, 2 * T], F32R, tag="h" + s)
                            nc.scalar.activation(h1[:], ps_in[:], AF.Relu,
                                                 bias=bap("bin_" + s), scale=1.0)

                        # ---- hidden layers ----
                        h = h1
                        for l in (1, 2):
                            FD = h.shape[1]
                            ps_l = pspool.tile([128, FD], F32, tag="ps")
                            for k in range(FD // T):
                                nc.tensor.matmul(ps_l[:, k * T:(k + 1) * T],
                                                 wap(f"sl{l}_{s}"),
                                                 h[:, k * T:(k + 1) * T],
                                                 start=True, stop=True)
                            h = residual(h, ps_l, l, s)

                        # ---- out-proj ----
                        ps_o = psopool.tile([mo, 2 * T], F32, tag="pso")
                        if s == "ins":
                            nc.tensor.matmul(ps_o[:, 0:T], wap("sout_ins")[0:64, :],
                                             h[0:64, :], start=True, stop=True)
                            nc.tensor.matmul(ps_o[:, T:2 * T],
                                             wap("sout_ins")[64:128, :],
                                             h[64:128, :], start=True, stop=True)
                        else:
                            nc.tensor.matmul(ps_o[:, 0:T], wap("sout_" + s),
                                             h[:, 0:T], start=True, stop=True)
                            nc.tensor.matmul(ps_o[:, T:2 * T], wap("sout_" + s),
                                             h[:, T:2 * T], start=True, stop=True)
                        # bias + copy into the staging tiles (A and B halves)
                        nc.scalar.activation(
                            o_t[(s, 0)][:, gl * T:(gl + 1) * T], ps_o[:, 0:T],
                            AF.Identity, bias=bap("bout_" + s, rows=mo), scale=1.0)
                        nc.scalar.activation(
                            o_t[(s, 1)][:, gl * T:(gl + 1) * T], ps_o[:, T:2 * T],
                            AF.Identity, bias=bap("bout_" + s, rows=mo), scale=1.0)

                # flush staging tiles: per section, A block then B block
                for s in SECS:
                    lo, hi = OCH[s]
                    nc.sync.dma_start(
                        out=out_d.ap()[lo:hi, f * GT * T:(f + 1) * GT * T],
                        in_=o_t[(s, 0)][:])
                    nc.sync.dma_start(
                        out=out_d.ap()[lo:hi, NG + f * GT * T:NG + (f + 1) * GT * T],
                        in_=o_t[(s, 1)][:])

    nc.compile()
    return nc


def _prep_inputs(inputs):
    f32 = np.float32

    def wT(name):
        return np.ascontiguousarray(np.asarray(inputs[name], f32).T)

    m = {}
    m["sin_co"] = np.concatenate([np.concatenate([wT("cls_Win"), wT("obj_Win")], 1)] * 2, 0)
    m["sin_bp"] = np.concatenate([np.concatenate([wT("box_Win"), wT("pos_Win")], 1)] * 2, 0)
    m["sin_ins"] = _bd(wT("ins_Win"), wT("ins_Win"))
    for l in (1, 2):
        m[f"sl{l}_co"] = _bd(np.asarray(inputs["cls_Wh"][l - 1], f32).T,
                             np.asarray(inputs["obj_Wh"][l - 1], f32).T)
        m[f"sl{l}_bp"] = _bd(np.asarray(inputs["box_Wh"][l - 1], f32).T,
                             np.asarray(inputs["pos_Wh"][l - 1], f32).T)
        m[f"sl{l}_ins"] = _bd(np.asarray(inputs["ins_Wh"][l - 1], f32).T,
                              np.asarray(inputs["ins_Wh"][l - 1], f32).T)
    m["sout_co"] = _bd(wT("cls_Wout"), wT("obj_Wout"))        # [128, 83]
    m["sout_bp"] = _bd(wT("box_Wout"), wT("pos_Wout"))        # [128, 68]
    m["sout_ins"] = np.concatenate([wT("ins_Wout")] * 2, 0)   # [128, 128]

    def colv(v):
        return np.asarray(v, f32).reshape(-1)

    m["bin_co"] = np.concatenate([colv(inputs["cls_bin"]), colv(inputs["obj_bin"])])
    m["bin_bp"] = np.concatenate([colv(inputs["box_bin"]), colv(inputs["pos_bin"])])
    m["bin_ins"] = np.concatenate([colv(inputs["ins_bin"])] * 2)
    for l in (1, 2):
        m[f"bh{l}_co"] = np.concatenate([colv(inputs["cls_bh"][l - 1]),
                                         colv(inputs["obj_bh"][l - 1])])
        m[f"bh{l}_bp"] = np.concatenate([colv(inputs["box_bh"][l - 1]),
                                         colv(inputs["pos_bh"][l - 1])])
        m[f"bh{l}_ins"] = np.concatenate([colv(inputs["ins_bh"][l - 1])] * 2)
    m["bout_co"] = np.concatenate([colv(inputs["cls_bout"]), colv(inputs["obj_bout"])])
    m["bout_bp"] = np.concatenate([colv(inputs["box_bout"]), colv(inputs["pos_bout"])])
    m["bout_ins"] = colv(inputs["ins_bout"])

    wp = np.zeros((128, WCOLS_TOTAL), f32)
    for name, (c0, n) in _W_COLS.items():
        v = m[name]
        if v.ndim == 1:
            wp[:v.shape[0], c0] = v
        else:
            wp[:v.shape[0], c0:c0 + n] = v
    wp = np.ascontiguousarray(wp)

    fast = all(not np.any(m[k]) for k in
               ["bh1_co", "bh1_bp", "bh1_ins", "bh2_co", "bh2_bp", "bh2_ins"])

    x = np.asarray(inputs["x"], f32)
    in_maps = []
    for c in range(NCORES):
        b, hh = c // 2, c % 2
        xs = x[b, :, hh * 128:(hh + 1) * 128, :].reshape(64, NPX)
        xsr = np.ascontiguousarray(
            np.concatenate([xs[:, :NG], xs[:, NG:]], axis=0))   # [128, NG]
        in_maps.append({"wp": wp, "xs": xsr})
    return in_maps, fast


def kernel(**inputs) -> np.ndarray:
    global _last_results
    in_maps, fast = _prep_inputs(inputs)
    if fast not in _cache:
        _cache[fast] = _build(fast)
    nc = _cache[fast]
    res = run_bass_kernel_spmd(nc, in_maps, core_ids=list(range(NCORES)))
    _last_results = res

    out = np.empty((B, OD, H, W), np.float32)
    for c in range(NCORES):
        b, hh = c // 2, c % 2
        out[b, :, hh * 128:(hh + 1) * 128, :] = \
            res.results[c]["out"].reshape(OD, 128, W)
    return out


# revision 9
# speedup vs baseline: 2.8787x; 2.8787x over previous
"""Trainium2 Bass kernel for the 5-head detection tower (nn_DFD_10849087390476).

Network (per head h of 5): 1x1-conv tower on x [B,64,H,W]:
    h1 = relu(Win x + bin)
    h2 = h1 + relu(Wh0 h1 + bh0)
    h3 = h2 + relu(Wh1 h2 + bh1)
    out_h = Wout h3 + bout
Output = concat over heads: channels [cls 81, obj 2, box 4, pos 64, ins 128] = 279.

Sharding: data-parallel over (batch, H/2) -> 8 shards of 32768 pixels.
Per core the pixel set is split in two 16384-px groups (A on SBUF partitions
0-63, B on 64-127). Heads are paired (cls+obj, box+pos) with block-diagonal
128x128 stationaries so every matmul uses the full partition dim; the ins head
rides the A/B stacking instead. Matmuls run in float32r (TF32-like, 1 cycle/row
at free-dim 512). Residuals are fused on the vector engine as max(psum,0)+h via
scalar_tensor_tensor; in-layer bias comes free via the scalar-engine activation
bias, out-layer bias via activation Identity+bias.

DMA layout: one packed weight DMA; x loaded in [128,4096] chunks; outputs
staged in SBUF for 8 pixel-tiles and flushed as [mo,4096] stores with
16KB-contiguous per-partition runs (keeps HWDGE descriptor count low).
"""
import numpy as np

from concourse import bacc, tile
import concourse.mybir as mybir
from concourse.bass_utils import run_bass_kernel_spmd

F32 = mybir.dt.float32
F32R = mybir.dt.float32r
AF = mybir.ActivationFunctionType
ALU = mybir.AluOpType

B, C, H, W = 4, 64, 256, 256
NCORES = 8
NPX = (B * H * W) // NCORES          # 32768 pixels per core
NG = NPX // 2                        # 16384 per group (A/B)
T = 512                              # pixels per matmul tile
NT = NG // T                         # 32 pair-tiles per core
GT = 4                               # pair-tiles per output flush group
NF = NT // GT                        # 4 flush groups
OD = 279                             # output channels

SECS = ("co", "bp", "ins")
MO = {"co": 83, "bp": 68, "ins": 128}
OCH = {"co": (0, 83), "bp": (83, 151), "ins": (151, 279)}

# packed weight-tensor column layout
_W_COLS = {}
_c = 0
for _n in ("sin_co", "sin_bp", "sin_ins", "sl1_co", "sl1_bp", "sl1_ins",
           "sl2_co", "sl2_bp", "sl2_ins"):
    _W_COLS[_n] = (_c, 128); _c += 128
for _s in SECS:
    _W_COLS["sout_" + _s] = (_c, MO[_s]); _c += MO[_s]
for _n in ("bin_co", "bin_bp", "bin_ins",
           "bh1_co", "bh1_bp", "bh1_ins", "bh2_co", "bh2_bp", "bh2_ins",
           "bout_co", "bout_bp", "bout_ins"):
    _W_COLS[_n] = (_c, 1); _c += 1
WCOLS_TOTAL = _c

_last_results = None                 # test.py reads exec_time_ns from here
_cache = {}


def _bd(a, b):
    out = np.zeros((a.shape[0] + b.shape[0], a.shape[1] + b.shape[1]), np.float32)
    out[:a.shape[0], :a.shape[1]] = a
    out[a.shape[0]:, a.shape[1]:] = b
    return out


def _build(fast: bool):
    nc = bacc.Bacc("TRN2", target_bir_lowering=False, debug=False)

    xs_d = nc.dram_tensor("xs", [128, NG], F32, kind="ExternalInput")
    wp_d = nc.dram_tensor("wp", [128, WCOLS_TOTAL], F32, kind="ExternalInput")
    out_d = nc.dram_tensor("out", [OD, NPX], F32, kind="ExternalOutput")

    with tile.TileContext(nc) as tc:
        with tc.tile_pool(name="const", bufs=1) as cpool, \
             tc.tile_pool(name="xp", bufs=2) as xpool, \
             tc.tile_pool(name="hp", bufs=3) as hpool, \
             tc.tile_pool(name="op", bufs=2) as opool, \
             tc.tile_pool(name="ps", bufs=1, space="PSUM") as pspool, \
             tc.tile_pool(name="pso", bufs=1, space="PSUM") as psopool:

            wp_t = cpool.tile([128, WCOLS_TOTAL], F32R, tag="wp")
            nc.sync.dma_start(out=wp_t[:], in_=wp_d.ap().bitcast(F32R))

            def wap(name):                      # stationary weight AP (f32r)
                c0, n = _W_COLS[name]
                return wp_t[:, c0:c0 + n]

            def bap(name, rows=128):            # bias AP (f32)
                c0, _ = _W_COLS[name]
                return wp_t[0:rows, c0:c0 + 1].bitcast(F32)

            def residual(h_prev, psum, l, s):
                """h_next = h_prev + relu(psum + bh)."""
                P, FD = h_prev.shape[0], h_prev.shape[1]
                h_next = hpool.tile([P, FD], F32R, tag=f"h{s}")
                if fast:
                    nc.vector.scalar_tensor_tensor(
                        h_next[:], psum[:], 0.0, h_prev[:], ALU.max, ALU.add)
                else:
                    r = hpool.tile([P, FD], F32, tag=f"r{s}")
                    nc.scalar.activation(r[:], psum[:], AF.Relu,
                                         bias=bap(f"bh{l}_{s}"), scale=1.0)
                    nc.vector.tensor_add(h_next[:], h_prev[:], r[:])
                return h_next

            def load_x(f):
                x_t = xpool.tile([128, GT * T], F32R, tag="x", name=f"x_{f}")
                nc.sync.dma_start(
                    out=x_t[:],
                    in_=xs_d.ap()[:, f * GT * T:(f + 1) * GT * T].bitcast(F32R))
                return x_t

            x_next = load_x(0)
            for f in range(NF):
                x_t = x_next
                if f + 1 < NF:
                    x_next = load_x(f + 1)

                o_t = {s: opool.tile([MO[s], 2 * GT * T], F32,
                                     tag=f"o{s}", name=f"o_{s}")
                       for s in SECS}

                for gl in range(GT):
                    xg = x_t[:, gl * T:(gl + 1) * T]
                    ps_in, h1 = {}, {}
                    # ---- in-proj matmuls ----
                    for s in SECS:
                        if s == "ins":
                            p = pspool.tile([128, T], F32, tag="psins", name="psins",
                                            bufs=2)
                            nc.tensor.matmul(p[:], wap("sin_ins"), xg,
                                             start=True, stop=True)
                        else:
                            p = pspool.tile([128, 2 * T], F32, tag="ps" + s,
                                            name="ps_" + s)
                            nc.tensor.matmul(p[:, 0:T], wap("sin_" + s)[0:64, :],
                                             xg[0:64, :], start=True, stop=True)
                            nc.tensor.matmul(p[:, T:2 * T],
                                             wap("sin_" + s)[64:128, :],
                                             xg[64:128, :], start=True, stop=True)
                        ps_in[s] = p
                    # ---- in relu ----
                    for s in SECS:
                        FD = ps_in[s].shape[1]
                        h1[s] = hpool.tile([128, FD], F32R, tag="h" + s,
                                           name="h1_" + s)
                        nc.scalar.activation(h1[s][:], ps_in[s][:], AF.Relu,
                                             bias=bap("bin_" + s), scale=1.0)
                    # ---- hidden layers, stage-major across sections ----
                    h = h1
                    for l in (1, 2):
                        ps_l = {}
                        for s in SECS:
                            FD = h[s].shape[1]
                            tg = "psins" if s == "ins" else "ps" + s
                            p = pspool.tile([128, FD], F32, tag=tg, name="psl_" + s,
                                            bufs=2 if s == "ins" else None)
                            for k in range(FD // T):
                                nc.tensor.matmul(p[:, k * T:(k + 1) * T],
                                                 wap(f"sl{l}_{s}"),
                                                 h[s][:, k * T:(k + 1) * T],
                                                 start=True, stop=True)
                            ps_l[s] = p
                        h = {s: residual(h[s], ps_l[s], l, s) for s in SECS}
                    # ---- out-proj ----
                    ps_o = {}
                    for s in SECS:
                        mo = MO[s]
                        p = psopool.tile([mo, 2 * T], F32, tag="pso",
                                         name="pso_" + s)
                        if s == "ins":
                            nc.tensor.matmul(p[:, 0:T], wap("sout_ins")[0:64, :],
                                             h[s][0:64, :], start=True, stop=True)
                            nc.tensor.matmul(p[:, T:2 * T],
                                             wap("sout_ins")[64:128, :],
                                             h[s][64:128, :], start=True, stop=True)
                        else:
                            nc.tensor.matmul(p[:, 0:T], wap("sout_" + s),
                                             h[s][:, 0:T], start=True, stop=True)
                            nc.tensor.matmul(p[:, T:2 * T], wap("sout_" + s),
                                             h[s][:, T:2 * T], start=True, stop=True)
                        ps_o[s] = p
                    # ---- out bias+copy (one 3D write per section) ----
                    for s in SECS:
                        mo = MO[s]
                        dst = o_t[s][:].rearrange("p (g c) -> p g c", g=2)[
                            :, :, gl * T:(gl + 1) * T]
                        nc.scalar.activation(dst, ps_o[s][:], AF.Identity,
                                             bias=bap("bout_" + s, rows=mo),
                                             scale=1.0)

                # flush staging tiles: per section, A block then B block
                for s in SECS:
                    lo, hi = OCH[s]
                    nc.gpsimd.dma_start(
                        out=out_d.ap()[lo:hi, f * GT * T:(f + 1) * GT * T],
                        in_=o_t[s][:, 0:GT * T])
                    nc.gpsimd.dma_start(
                        out=out_d.ap()[lo:hi, NG + f * GT * T:NG + (f + 1) * GT * T],
                        in_=o_t[s][:, GT * T:2 * GT * T])

    nc.compile()
    return nc


def _prep_inputs(inputs):
    f32 = np.float32

    def wT(name):
        return np.ascontiguousarray(np.asarray(inputs[name], f32).T)

    m = {}
    m["sin_co"] = np.concatenate([np.concatenate([wT("cls_Win"), wT("obj_Win")], 1)] * 2, 0)
    m["sin_bp"] = np.concatenate([np.concatenate([wT("box_Win"), wT("pos_Win")], 1)] * 2, 0)
    m["sin_ins"] = _bd(wT("ins_Win"), wT("ins_Win"))
    for l in (1, 2):
        m[f"sl{l}_co"] = _bd(np.asarray(inputs["cls_Wh"][l - 1], f32).T,
                             np.asarray(inputs["obj_Wh"][l - 1], f32).T)
        m[f"sl{l}_bp"] = _bd(np.asarray(inputs["box_Wh"][l - 1], f32).T,
                             np.asarray(inputs["pos_Wh"][l - 1], f32).T)
        m[f"sl{l}_ins"] = _bd(np.asarray(inputs["ins_Wh"][l - 1], f32).T,
                              np.asarray(inputs["ins_Wh"][l - 1], f32).T)
    m["sout_co"] = _bd(wT("cls_Wout"), wT("obj_Wout"))        # [128, 83]
    m["sout_bp"] = _bd(wT("box_Wout"), wT("pos_Wout"))        # [128, 68]
    m["sout_ins"] = np.concatenate([wT("ins_Wout")] * 2, 0)   # [128, 128]

    def colv(v):
        return np.asarray(v, f32).reshape(-1)

    m["bin_co"] = np.concatenate([colv(inputs["cls_bin"]), colv(inputs["obj_bin"])])
    m["bin_bp"] = np.concatenate([colv(inputs["box_bin"]), colv(inputs["pos_bin"])])
    m["bin_ins"] = np.concatenate([colv(inputs["ins_bin"])] * 2)
    for l in (1, 2):
        m[f"bh{l}_co"] = np.concatenate([colv(inputs["cls_bh"][l - 1]),
                                         colv(inputs["obj_bh"][l - 1])])
        m[f"bh{l}_bp"] = np.concatenate([colv(inputs["box_bh"][l - 1]),
                                         colv(inputs["pos_bh"][l - 1])])
        m[f"bh{l}_ins"] = np.concatenate([colv(inputs["ins_bh"][l - 1])] * 2)
    m["bout_co"] = np.concatenate([colv(inputs["cls_bout"]), colv(inputs["obj_bout"])])
    m["bout_bp"] = np.concatenate([colv(inputs["box_bout"]), colv(inputs["pos_bout"])])
    m["bout_ins"] = colv(inputs["ins_bout"])

    wp = np.zeros((128, WCOLS_TOTAL), f32)
    for name, (c0, n) in _W_COLS.items():
        v = m[name]
        if v.ndim == 1:
            wp[:v.shape[0], c0] = v
        else:
            wp[:v.shape[0], c0:c0 + n] = v
    wp = np.ascontiguousarray(wp)

    fast = all(not np.any(m[k]) for k in
               ["bh1_co", "bh1_bp", "bh1_ins", "bh2_co", "bh2_bp", "bh2_ins"])

    x = np.asarray(inputs["x"], f32)
    in_maps = []
    for c in range(NCORES):
        b, hh = c // 2, c % 2
        xs = x[b, :, hh * 128:(hh + 1) * 128, :].reshape(64, NPX)
        xsr = np.ascontiguousarray(
            np.concatenate([xs[:, :NG], xs[:, NG:]], axis=0))   # [128, NG]
        in_maps.append({"wp": wp, "xs": xsr})
    return in_maps, fast


def kernel(**inputs) -> np.ndarray:
    global _last_results
    in_maps, fast = _prep_inputs(inputs)
    if fast not in _cache:
        _cache[fast] = _build(fast)
    nc = _cache[fast]
    res = run_bass_kernel_spmd(nc, in_maps, core_ids=list(range(NCORES)))
    _last_results = res

    out = np.empty((B, OD, H, W), np.float32)
    for c in range(NCORES):
        b, hh = c // 2, c % 2
        out[b, :, hh * 128:(hh + 1) * 128, :] = \
            res.results[c]["out"].reshape(OD, 128, W)
    return out
